# revision 68
# baseline (speedup 1.0000x reference)
"""Trainium2 Bass kernel for a dense transformer block (v4).

Problem: B=8, T=2048, DIM=384, 6 heads (hd=64), FFN hidden 768, causal
attention, RMSNorm (eps 1e-6), exact GELU, fp32 I/O.

Sharding: data-parallel over batch B=8 -> one batch element per NeuronCore,
no collectives. Each core runs the full block on its [2048, 384] slice.

v4 over v3 (337us -> 299 -> ~261us): PE is the binding engine (~207us
busy incl HAM cold-clock inflation); changes target PE density and the
latency-critical prologue:
  - inputs as few large DMAs split across BOTH HWDGE rings (x on sync,
    weights on scalar) so they land in parallel; x tiles 0-3 as individual
    transfers alternating rings (tile-0 latency gates the prologue);
  - a ~7.7us PE warmup spin (72 back-to-back matmuls vs identity) so HAM
    un-throttles 1.2->2.4GHz before the real pipeline head arrives;
  - QK matmuls skip the causally-dead column prefix of boundary tiles;
  - attention k-loop software-pipelined (QK(k+1) + dripped slack units
    emitted BEFORE AV(k), which blocks on exp(k)) and the inter-chunk
    slack work (norm1/produce/xwo/norm2) decomposed into small units
    dripped one-per-k-tile so the in-order PE queue never micro-idles
    while ACT chews the exp stream (keeps HAM warm);
  - o-evacuation fused: o_sb = o_ps * bcast(1/Z) straight out of PSUM;
  - 1 Newton step in the quake rsqrt (DVE chain latency ~halves; ~1e-3
    rel on the norm scale, fine at the 2e-2 gate).
Measured dead ends (reverted): fp8e4 anywhere costs 2-3e-2 on the
max-abs-err metric (DoubleRow AV measured 1.9e-2, sim confirms all fp8
paths land 2-4e-2 -> over budget); FFN moved under the attention window
stalls the strict-FIFO ACT queue head-of-line (gelu waiting on PE blocks
later exps) and regressed to 297-416us in three different shapes.

v3 design (unchanged core):
  - Attention: S^T layout, Tq=512 chunks, one [128,1024] S tile per
    (feature-pair, ktile) holding both heads of the pair; QK matmuls
    contract K=64 and run row-tiled (tile_position (0,0)/(64,0)) so the
    two heads compute CONCURRENTLY into the two banks; one exp covers
    both. AV keeps the ones-column normalizer (M=65) with the ones FIRST
    so Z lands on PSUM partition 0 (no cross-partition hop for the
    reciprocal/broadcast chain).
  - RMSNorm entirely off ScalarE: sum(x^2) via DVE scalar_tensor_tensor
    accum, rsqrt via the fp32 magic-constant seed + 2 Newton steps on DVE
    int/float ops. ScalarE runs ONLY exp and gelu -> 2 table loads total.
  - Causal diagonals: exp skips the fully-dead prefix (2D strided AP),
    gpsimd memsets zero the prefix in p, and a narrowed DVE multiply with
    a precomputed 0/1 band masks the triangle.
  - PE transposes are regular bf16 matmuls vs identity (N=128, pipelined,
    HAM-warm) into one PSUM bank; single strided DVE copy evacuates all
    3 chunks into fused feature-major tiles.
  - h = x*rsqrt scaling runs on gpsimd (tensor_scalar, 1-input line rate)
    to keep DVE headroom.
  - Emission is chunk-pipelined ascending: attention for chunk ch starts
    after its own K/Q columns; x+o@wo, norm2, second transpose of chunk
    ch overlap attention of chunk ch+1. FFN (gelu-gated) runs as a tail.
  - PSUM: 2 (shared proj/transpose) + 4 (S x2) + 2 (o_even/o_odd) = 8.
"""

import math
import sys

import ml_dtypes
import numpy as np

for _p in ("/opt/trn_rl_repo",):
    if _p not in sys.path:
        sys.path.append(_p)

import concourse.bacc as bacc
import concourse.bass as bass
import concourse.mybir as mybir
import concourse.tile as tile
from concourse.bass import ts
from concourse.bass_utils import run_bass_kernel_spmd
from concourse.masks import make_identity

F32 = mybir.dt.float32
BF16 = mybir.dt.bfloat16
FP8 = mybir.dt.float8e4
I32 = mybir.dt.int32
AF = mybir.ActivationFunctionType
ALU = mybir.AluOpType
DR = mybir.MatmulPerfMode.DoubleRow
EXPB = -2.0794415416798357  # -ln(8): keeps exp() in fp8e4 range; Z scales too

NCORES = 8
T, D, NH, HD, HDIM = 2048, 384, 6, 64, 768
P = 128
SLOT = 128             # per-head V slot: [ones, 63 zeros, v_0..v_63]
VOFF = 64              # v columns live at [VOFF, VOFF+HD); o rows 64-aligned
NT = T // P            # 16 token tiles
ND = D // P            # 3 feature chunks
NHT = HDIM // P        # 6 FFN hidden chunks
CH = 512               # Tq chunk width
NCH = T // CH          # 4
EPS = 1e-6
SCL = 1.0 / math.sqrt(HD)
MAGIC = 0x5F3759DF


def _body(tc, din, out_d):
    nc = tc.nc

    main_cm = tc.tile_pool(name="main", bufs=1)
    main = main_cm.__enter__()

    # ---- big consolidated input tiles (few, large DMAs; split across the
    # two HWDGE queues so x and weights land in parallel) ----
    xbig0 = main.tile([P, 4 * D], F32, tag="xb0", name="xb0")
    xbig1 = main.tile([P, 12 * D], F32, tag="xb1", name="xb1")
    wq_all = main.tile([P, ND * D], BF16, tag="wqa", name="wqa")
    wk_all = main.tile([P, ND * D], BF16, tag="wka", name="wka")
    wv_all = main.tile([P, ND * D], BF16, tag="wva", name="wva")
    wo_all = main.tile([P, ND * D], BF16, tag="woa", name="woa")
    fw1_all = main.tile([P, ND * HDIM], BF16, tag="f1a", name="f1a")
    fw2_all = main.tile([P, NHT * D], BF16, tag="f2a", name="f2a")

    # critical first: x tiles 0-3 as individual transfers (latency of tile 0
    # gates the whole prologue) on sync queue; QKV weights on scalar queue
    for j in range(4):
        eng = nc.sync if j % 2 == 0 else nc.scalar
        eng.dma_start(xbig0[:, ts(j, D)], din["x"][ts(j, P), :])
    nc.scalar.dma_start(
        wk_all[:].rearrange("p (c d) -> p c d", c=ND),
        din["wk"].rearrange("(c p) d -> p c d", p=P))
    nc.scalar.dma_start(
        wq_all[:].rearrange("p (c d) -> p c d", c=ND),
        din["wq"].rearrange("(c p) d -> p c d", p=P))
    nc.scalar.dma_start(
        wv_all[:].rearrange("p (c d) -> p c d", c=ND),
        din["wv"].rearrange("(c p) d -> p c d", p=P))
    nc.sync.dma_start(
        xbig1[:].rearrange("p (j d) -> p j d", j=12),
        din["x"][512:T, :].rearrange("(j p) d -> p j d", p=P))

    # ---- constants ----
    ident = main.tile([P, P], BF16, tag="ident", name="ident")
    make_identity(nc, ident[:])

    # PE warmup spin: ~5us of back-to-back matmuls so HAM un-throttles the
    # clock (1.2 -> 2.4 GHz) before the real pipeline head arrives.
    warm_cm = tc.tile_pool(name="warm", bufs=1, space="PSUM")
    warm = warm_cm.__enter__()
    wtile = warm.tile([P, P], F32, tag="wrm", name="wrm")
    for _ in range(72):
        nc.tensor.matmul(wtile[:], ident[:], ident[:], start=True, stop=True)
    warm_cm.__exit__(None, None, None)
    onesf = main.tile([P, P], F32, tag="onesf", name="onesf")
    nc.gpsimd.memset(onesf[:], 1.0)
    ones_bf = main.tile([1, P], BF16, tag="ones", name="ones")
    nc.vector.tensor_copy(ones_bf[:], onesf[0:1, :])
    magic_t = main.tile([P, 16], I32, tag="magic", name="magic")
    nc.gpsimd.memset(magic_t[:], MAGIC)
    # band[k, c] = 1 iff c - k >= CH (0/1 mask for causal diagonals),
    # built directly in bf16 (no f32 staging tile -- frees 4KB SBUF for
    # the 6th p-tile buffer)
    band = main.tile([P, 2 * CH], BF16, tag="band", name="band")
    nc.gpsimd.memset(band[:], 1.0)
    nc.gpsimd.affine_select(out=band[:], in_=band[:],
                            compare_op=ALU.is_ge, fill=0.0,
                            base=-CH, channel_multiplier=-1,
                            pattern=[[1, 2 * CH]])

    s1 = main.tile([P, NT], F32, tag="s1", name="s1")
    s1i = main.tile([P, NT], F32, tag="s1i", name="s1i")
    s2 = main.tile([P, NT], F32, tag="s2", name="s2")
    s2i = main.tile([P, NT], F32, tag="s2i", name="s2i")

    # ---- big feature-major tensors (single tiles; chunk c = cols c*T..) ----
    ht = main.tile([P, ND * T], BF16, tag="ht", name="ht")
    qt = main.tile([P, ND * T], BF16, tag="qt", name="qt")
    kt = main.tile([P, ND * T], BF16, tag="kt", name="kt")
    ot = main.tile([P, ND * T], BF16, tag="ot", name="ot")
    h2t = main.tile([P, ND * T], BF16, tag="h2t", name="h2t")
    gt = main.tile([P, NHT * T], BF16, tag="gt", name="gt")

    x_tiles = ([xbig0[:, ts(j, D)] for j in range(4)]
               + [xbig1[:, ts(j, D)] for j in range(12)])
    vaug = [main.tile([P, NH * SLOT], BF16, tag=f"va{j}", name=f"va{j}")
            for j in range(NT)]

    # ---- weight slice views ----
    wq_s = [wq_all[:, ts(c, D)] for c in range(ND)]
    wk_s = [wk_all[:, ts(c, D)] for c in range(ND)]
    wv_s = [wv_all[:, ts(c, D)] for c in range(ND)]
    wo_s = [wo_all[:, ts(c, D)] for c in range(ND)]
    fw1_s = [fw1_all[:, ts(c, HDIM)] for c in range(ND)]
    fw2_s = [fw2_all[:, ts(c, D)] for c in range(NHT)]
    b1_s = main.tile([P, NHT], F32, tag="b1", name="b1")
    b2_row = main.tile([1, D], BF16, tag="b2", name="b2")
    b2b = main.tile([P, D], BF16, tag="b2b", name="b2b")

    def dma_bulk():
        nc.scalar.dma_start(
            wo_all[:].rearrange("p (c d) -> p c d", c=ND),
            din["wo"].rearrange("(c p) d -> p c d", p=P))
        nc.scalar.dma_start(
            fw1_all[:].rearrange("p (c h) -> p c h", c=ND),
            din["fw1"].rearrange("(c p) h -> p c h", p=P))
        nc.scalar.dma_start(
            fw2_all[:].rearrange("p (c d) -> p c d", c=NHT),
            din["fw2"].rearrange("(c p) d -> p c d", p=P))
        nc.sync.dma_start(b1_s[:], din["fb1"].rearrange("(a b) -> b a", b=P))
        nc.sync.dma_start(b2_row[:], din["fb2"].rearrange("(a b) -> a b", a=1))
        nc.gpsimd.partition_broadcast(b2b[:], b2_row[0:1, :])

    # per-head V slots: ones col 0 (Z -> PSUM row 0), zeros, v at 64:128
    for j in range(NT):
        nc.gpsimd.memset(vaug[j][:], 0.0)
        nc.gpsimd.memset(
            vaug[j][:].rearrange("p (h e) -> p h e", h=NH)[:, :, 0:1], 1.0)

    # ---- scratch pools ----
    pscr_cm = tc.tile_pool(name="scr", bufs=4)
    pscr = pscr_cm.__enter__()
    prs_cm = tc.tile_pool(name="rsq", bufs=2)
    prs = prs_cm.__enter__()
    patt_cm = tc.tile_pool(name="att", bufs=6)
    patt = patt_cm.__enter__()
    pnrm_cm = tc.tile_pool(name="nrm", bufs=3)
    pnrm = pnrm_cm.__enter__()
    pout_cm = tc.tile_pool(name="out", bufs=3)
    pout = pout_cm.__enter__()
    pg_cm = tc.tile_pool(name="gb", bufs=12)
    pg = pg_cm.__enter__()

    pj_cm = tc.tile_pool(name="pj", bufs=2, space="PSUM")
    pj = pj_cm.__enter__()

    def rsqrt_quake(s_acc, s_inv, j0, n):
        """s_inv[:, j0:j0+n] = 1/sqrt(s_acc[:, j0:j0+n]/D + EPS) on DVE."""
        tq = prs.tile([P, 16], F32, tag="tq", name="tq")
        sc = prs.tile([P, 16], F32, tag="sc", name="sc")
        y0 = prs.tile([P, 16], F32, tag="y0", name="y0")
        y1 = prs.tile([P, 16], F32, tag="y1", name="y1")
        t_ = tq[:, 0:n]
        nc.vector.tensor_scalar(t_, s_acc[:, j0 : j0 + n], 1.0 / D, EPS,
                                op0=ALU.mult, op1=ALU.add)
        # seed: y0 = bitcast(MAGIC - (bitcast_i32(t) >> 1))
        nc.vector.tensor_scalar(y0[:, 0:n].bitcast(I32), t_.bitcast(I32),
                                1, None, op0=ALU.arith_shift_right)
        nc.vector.scalar_tensor_tensor(
            y0[:, 0:n].bitcast(I32), magic_t[:, 0:n], 0,
            y0[:, 0:n].bitcast(I32), op0=ALU.bypass, op1=ALU.subtract)
        # one Newton step: y <- y * (1.5 - 0.5 * t * y^2)  (~1e-3 rel, fine
        # at the 2e-2 gate; halves the serial DVE latency on the norm chain)
        for src, dst in ((y0, None),):
            out_ap = s_inv[:, j0 : j0 + n] if dst is None else dst[:, 0:n]
            nc.vector.tensor_mul(sc[:, 0:n], src[:, 0:n], src[:, 0:n])
            nc.vector.tensor_mul(sc[:, 0:n], sc[:, 0:n], t_)
            nc.vector.tensor_scalar(sc[:, 0:n], sc[:, 0:n], -0.5, 1.5,
                                    op0=ALU.mult, op1=ALU.add)
            nc.vector.tensor_mul(out_ap, src[:, 0:n], sc[:, 0:n])

    def stats_rsqrt(js, s_acc, s_inv):
        """RMSNorm stats (DVE STT accum) + quake rsqrt for a 4-tile group."""
        for j in js:
            sq = pscr.tile([P, D], F32, tag="sq", name="sq")
            nc.vector.scalar_tensor_tensor(
                sq[:], x_tiles[j][:], 1.0, x_tiles[j][:],
                op0=ALU.mult, op1=ALU.mult,
                accum_out=s_acc[:, j : j + 1])
        rsqrt_quake(s_acc, s_inv, js[0], len(js))

    def scale_transpose(j, s_inv, dst, via_dma=True):
        """DVE scale by rsqrt + PE transpose + evac for one token tile."""
        hb = pscr.tile([P, D], BF16, tag="hb", name="hb")
        nc.vector.tensor_scalar_mul(hb[:], x_tiles[j][:],
                                    s_inv[:, j : j + 1])
        tp = pj.tile([P, CH], F32, tag="pj", name="tp")
        for c in range(ND):
            nc.tensor.matmul(tp[:, ts(c, P)], hb[:, ts(c, P)], ident[:],
                             start=True, stop=True)
        dstv = dst[:].rearrange("p (c t) -> p c t", c=ND)[:, :, ts(j, P)]
        if via_dma:
            # contiguous bf16 evac (2x DVE), then DMA-scatter to chunks
            tb = pscr.tile([P, D], BF16, tag="tb", name="tb")
            nc.vector.tensor_copy(tb[:], tp[:, 0:D])
            nc.sync.dma_start(
                dstv, tb[:].rearrange("p (c t) -> p c t", c=ND))
        else:
            # latency-critical prologue: strided copy, no DMA hop
            nc.vector.tensor_copy(
                dstv, tp[:, 0:D].rearrange("p (c t) -> p c t", c=ND))

    def norm_and_transpose(js, s_acc, s_inv, dst, via_dma=True):
        """RMSNorm (DVE stats + quake rsqrt), DVE scale, PE transpose."""
        stats_rsqrt(js, s_acc, s_inv)
        for j in js:
            scale_transpose(j, s_inv, dst, via_dma)

    def kq_unit(w_s, dstt, dt2, ch2, via_dma=True):
        """One K^T or Q^T projection column block for chunk ch2."""
        ps = pj.tile([P, CH], F32, tag="pj", name="kq")
        for c in range(ND):
            nc.tensor.matmul(
                ps[:], w_s[c][:, ts(dt2, P)],
                ht[:, c * T + ch2 * CH : c * T + ch2 * CH + CH],
                start=(c == 0), stop=(c == ND - 1))
        nc.vector.tensor_copy(
            dstt[:, dt2 * T + ch2 * CH : dt2 * T + ch2 * CH + CH], ps[:])

    def v_unit(j, via_dma=True):
        """V projection + slot scatter for one token tile."""
        ps = pj.tile([P, CH], F32, tag="pj", name="v")
        for c in range(ND):
            nc.tensor.matmul(
                ps[:, 0:D], ht[:, c * T + j * P : c * T + (j + 1) * P],
                wv_s[c][:], start=(c == 0), stop=(c == ND - 1))
        dstv = vaug[j][:].rearrange("p (h e) -> p h e",
                                    h=NH)[:, :, VOFF : VOFF + HD]
        if via_dma:
            vb = pscr.tile([P, D], BF16, tag="vb", name="vb")
            nc.vector.tensor_copy(vb[:], ps[:, 0:D])
            nc.sync.dma_start(
                dstv, vb[:].rearrange("p (h e) -> p h e", h=NH))
        else:
            nc.vector.tensor_copy(
                dstv, ps[:, 0:D].rearrange("p (h e) -> p h e", h=NH))

    def produce_kqv(ch, via_dma=True):
        """K^T/Q^T columns + V slots for chunk ch."""
        for w_s, dstt in ((wk_s, kt), (wq_s, qt)):
            for dt in range(ND):
                kq_unit(w_s, dstt, dt, ch, via_dma)
        for j in range(4 * ch, 4 * ch + 4):
            v_unit(j, via_dma)

    def xwo_unit(j):
        ps = pj.tile([P, CH], F32, tag="pj", name="xo")
        for dt in range(ND):
            nc.tensor.matmul(
                ps[:, 0:D], ot[:, dt * T + j * P : dt * T + (j + 1) * P],
                wo_s[dt][:], start=(dt == 0), stop=(dt == ND - 1))
        nc.vector.tensor_add(x_tiles[j][:], ps[:, 0:D], x_tiles[j][:])

    def xwo_chunk(ch):
        """x2 = x + o @ wo for chunk ch's token tiles."""
        for j in range(4 * ch, 4 * ch + 4):
            xwo_unit(j)

    def ffn1_unit(c, hti):
        """One FFN1 hidden block for chunk c: matmuls into pj, immediate
        DVE drain to SBUF (PSUM bank frees without waiting on ACT), gelu
        SBUF->SBUF queued on ACT for whenever it gets there."""
        g_ps = pj.tile([P, CH], F32, tag="pj", name="g")
        for cc in range(ND):
            nc.tensor.matmul(
                g_ps[:], fw1_s[cc][:, ts(hti, P)],
                h2t[:, cc * T + c * CH : cc * T + c * CH + CH],
                start=(cc == 0), stop=(cc == ND - 1))
        gb = pg.tile([P, CH], BF16, tag="gb", name="gb")
        nc.vector.tensor_copy(gb[:], g_ps[:])
        nc.scalar.activation(
            gt[:, hti * T + c * CH : hti * T + c * CH + CH],
            gb[:], AF.Gelu, bias=b1_s[:, hti : hti + 1])

    def ffn1_chunk(c):
        for hti in range(NHT):
            ffn1_unit(c, hti)

    def ffn2_unit(j):
        """FFN second half + residual + output store for token tile j.
        b2 is added on DVE (pre-broadcast tile) instead of a K=1 PE matmul
        -- PE is the binding engine, DVE has slack."""
        ps = pj.tile([P, CH], F32, tag="pj", name="f2")
        for cc in range(NHT):
            nc.tensor.matmul(
                ps[:, 0:D], gt[:, cc * T + j * P : cc * T + (j + 1) * P],
                fw2_s[cc][:], start=(cc == 0), stop=(cc == NHT - 1))
        o_t = pout.tile([P, D], F32, tag="ot", name="otl")
        nc.vector.tensor_add(o_t[:], ps[:, 0:D], x_tiles[j][:])
        nc.vector.tensor_add(o_t[:], o_t[:], b2b[:])
        nc.sync.dma_start(out_d[ts(j, P), :], o_t[:])

    def ffn2_chunk(c):
        for j in range(4 * c, 4 * c + 4):
            ffn2_unit(j)

    # ---- prologue: norm1 + K/Q/V for chunk 0 (no DMA hops) ----
    norm_and_transpose(range(4), s1, s1i, ht, via_dma=False)

    psS_cm = tc.tile_pool(name="psS", bufs=2, space="PSUM")
    psS = psS_cm.__enter__()
    psO_cm = tc.tile_pool(name="psO", bufs=1, space="PSUM")
    psO = psO_cm.__enter__()

    # chunk-0 attention only needs the dt0 K/Q columns + V slots to start;
    # the dt1/dt2 projections drip into the dt0 k-loop below
    kq_unit(wk_s, kt, 0, 0)
    kq_unit(wq_s, qt, 0, 0)
    for j in range(4):
        v_unit(j, via_dma=False)
    dma_bulk()

    for ch in range(NCH):
        js = range(4 * ch, 4 * ch + 4)
        ntk = 4 * (ch + 1)
        # slack work for this chunk, decomposed into small units that are
        # dripped into the k-loop (one every few k-tiles) so PE never
        # micro-idles while ACT runs the exp stream (keeps HAM warm):
        units = []
        if ch == 0:
            # remainder of chunk-0's own projections (dt1/dt2)
            for d2 in (1, 2):
                units.append(lambda d2=d2: kq_unit(wk_s, kt, d2, 0))
                units.append(lambda d2=d2: kq_unit(wq_s, qt, d2, 0))
        if ch >= 1:
            pjs = range(4 * ch - 4, 4 * ch)
            units += [lambda j=j: xwo_unit(j) for j in pjs]
        if ch < NCH - 1:
            nj = range(4 * ch + 4, 4 * ch + 8)
            if ch == 0:
                units.append(lambda: stats_rsqrt(range(4, 8), s1, s1i))
            units += [lambda j=j: scale_transpose(j, s1i, ht) for j in nj]
            for w_s_, dstt_ in ((wk_s, kt), (wq_s, qt)):
                units += [lambda w=w_s_, dd=dstt_, d2=d2: kq_unit(w, dd, d2,
                                                                 ch + 1)
                          for d2 in range(ND)]
            units += [lambda j=j: v_unit(j) for j in nj]
            if ch <= 1:
                # hoist the NEXT-next chunk's norm1 stats (they only need x)
                # one chunk ahead, so no chunk boundary ever waits on the
                # DVE stats->rsqrt chain
                nnj = range(4 * ch + 8, 4 * ch + 12)
                units.append(lambda nnj=nnj: stats_rsqrt(nnj, s1, s1i))
        if ch >= 1:
            units.append(lambda pjs=pjs: stats_rsqrt(pjs, s2, s2i))
            units += [lambda j=j: scale_transpose(j, s2i, h2t) for j in pjs]
        if ch == NCH - 1:
            # ch3 is ACT-bound with few real slack units left; weave cheap
            # dummy matmuls between the real units so the PE never sits
            # idle long enough for HAM to re-throttle the clock (cold ch3
            # matmuls cost ~2x)
            def dummy_unit():
                t = pj.tile([P, CH], F32, tag="pj", name="dmy")
                nc.tensor.matmul(t[:, 0:P], ident[:], ident[:],
                                 start=True, stop=True)
            woven = []
            for u in units:
                woven += [u, dummy_unit, dummy_unit, dummy_unit]
            units = woven + [dummy_unit] * 9
        total_slots = ND * ntk
        emitted = 0

        def drip(slot):
            nonlocal emitted
            want = len(units) * (slot + 1) // total_slots
            while emitted < want:
                units[emitted]()
                emitted += 1

        for dt in range(ND):
            o_e = psO.tile([P, CH], F32, tag="oe", name="oe")
            o_o = psO.tile([P, CH], F32, tag="oo", name="oo")

            def qk_exp(k):
                """QK matmuls + exp + triangle mask for k-tile k; returns
                the p tile."""
                b = k - 4 * ch
                d = max(0, b) * P
                s_ps = psS.tile([P, 2 * CH], F32, tag="s", name="s")
                nc.tensor.matmul(
                    s_ps[:, d:CH],
                    kt[0:HD, dt * T + k * P : dt * T + (k + 1) * P],
                    qt[0:HD, dt * T + ch * CH + d : dt * T + ch * CH + CH],
                    start=True, stop=True, tile_position=(0, 0))
                nc.tensor.matmul(
                    s_ps[:, CH + d : 2 * CH],
                    kt[HD:P, dt * T + k * P : dt * T + (k + 1) * P],
                    qt[HD:P, dt * T + ch * CH + d : dt * T + ch * CH + CH],
                    start=True, stop=True, tile_position=(HD, 0))
                p_sb = patt.tile([P, 2 * CH], BF16, tag="p", name="p")
                if d == 0:
                    nc.scalar.activation(p_sb[:], s_ps[:], AF.Exp, scale=SCL)
                else:
                    # skip the fully-dead prefix of each parity half; the AV
                    # matmuls below only read q >= d so it stays unwritten
                    nc.scalar.activation(
                        p_sb[:].rearrange("p (v q) -> p v q", v=2)[:, :, d:CH],
                        s_ps[:].rearrange("p (v q) -> p v q", v=2)[:, :, d:CH],
                        AF.Exp, scale=SCL)
                if b >= 0:
                    # 128-wide boundary window gets the triangle mask
                    for par in range(2):
                        nc.vector.tensor_mul(
                            p_sb[:, par * CH + d : par * CH + d + P],
                            p_sb[:, par * CH + d : par * CH + d + P],
                            band[:, CH : CH + P])
                return p_sb

            def av(k, p_sb):
                d = max(0, k - 4 * ch) * P
                nc.tensor.matmul(
                    o_e[0:SLOT, d:CH],
                    vaug[k][:, (2 * dt) * SLOT : (2 * dt + 1) * SLOT],
                    p_sb[:, d:CH],
                    start=(k == 0), stop=(k == ntk - 1))
                nc.tensor.matmul(
                    o_o[0:SLOT, d:CH],
                    vaug[k][:, (2 * dt + 1) * SLOT : (2 * dt + 2) * SLOT],
                    p_sb[:, CH + d : 2 * CH],
                    start=(k == 0), stop=(k == ntk - 1))

            # software-pipelined: emit QK(k+1) BEFORE AV(k) so the in-order
            # PE queue always has the next S matmuls to chew on while ACT
            # runs exp(k); AV(k) then lands right after exp(k) completes.
            # Slack units drip in at per-k granularity.
            p_prev = qk_exp(0)
            drip(dt * ntk)
            for k in range(1, ntk):
                p_cur = qk_exp(k)
                # fill the PE queue here: av(k-1) blocks on exp(k-1), so the
                # dripped matmuls run while ACT chews the exp stream
                drip(dt * ntk + k)
                av(k - 1, p_prev)
                p_prev = p_cur
            av(ntk - 1, p_prev)
            # evacuate + normalize both heads (Z on PSUM row 0; o rows
            # 64:128). The PSUM bank is read by just two quick DVE ops
            # (recip + raw copy) so it frees ~1.5us earlier for the next
            # head-pair's AV(k=0); the normalize runs from SBUF after.
            for par, o_ps in ((0, o_e), (1, o_o)):
                zf = pnrm.tile([P, CH], F32, tag="zf", name="zf")
                nc.vector.reciprocal_approx_fast(zf[0:1, :], o_ps[0:1, :])
                o_raw = pnrm.tile([P, CH], BF16, tag="oraw", name="oraw")
                nc.vector.tensor_copy(o_raw[VOFF:SLOT, :], o_ps[VOFF:SLOT, :])
                zb = pnrm.tile([P, CH], BF16, tag="zb", name="zb")
                nc.vector.tensor_copy(zb[0:1, :], zf[0:1, :])
                zbb = pnrm.tile([P, CH], BF16, tag="zbb", name="zbb")
                nc.gpsimd.partition_broadcast(zbb[0:SLOT, :], zb[0:1, :])
                o_sb = pnrm.tile([P, CH], BF16, tag="osb", name="osb")
                nc.vector.tensor_mul(o_sb[VOFF:SLOT, :], o_raw[VOFF:SLOT, :],
                                     zbb[VOFF:SLOT, :])
                hp = par * HD
                if dt == ND - 1:
                    # last dt's o gates the next chunk's xwo units (and the
                    # tail): a DVE copy lands ~2.6us sooner than the DMA's
                    # HBM-receipt-latency path
                    nc.vector.tensor_copy(
                        ot[hp : hp + HD,
                           dt * T + ch * CH : dt * T + ch * CH + CH],
                        o_sb[VOFF:SLOT, :])
                else:
                    nc.sync.dma_start(
                        ot[hp : hp + HD,
                           dt * T + ch * CH : dt * T + ch * CH + CH],
                        o_sb[VOFF:SLOT, :])

        # flush any units not yet dripped (shouldn't happen, but safe)
        while emitted < len(units):
            units[emitted]()
            emitted += 1

    psO_cm.__exit__(None, None, None)
    psS_cm.__exit__(None, None, None)

    # ---- tail: norm2(3) DVE chain hidden under FFN1-h2=0 PE work ----
    # (h2=0 covers token chunks 0-1, which need nothing from chunk 3;
    # h2 OUTER gelus let FFN2 for chunks 0-1 start after half the stream)
    psF_cm = tc.tile_pool(name="psF", bufs=3, space="PSUM")
    psF = psF_cm.__enter__()

    def ffn1_h2(h2):
        for hti in range(NHT):
            g_ps = psF.tile([P, 2 * CH], F32, tag="g", name="g")
            for c in range(ND):
                nc.tensor.matmul(
                    g_ps[:, 0:CH], fw1_s[c][:, ts(hti, P)],
                    h2t[:, c * T + 2 * h2 * CH : c * T + (2 * h2 + 1) * CH],
                    start=(c == 0), stop=(c == ND - 1))
            for c in range(ND):
                nc.tensor.matmul(
                    g_ps[:, CH : 2 * CH], fw1_s[c][:, ts(hti, P)],
                    h2t[:, c * T + (2 * h2 + 1) * CH
                        : c * T + (2 * h2 + 2) * CH],
                    start=(c == 0), stop=(c == ND - 1))
            nc.scalar.activation(
                gt[:, hti * T + h2 * 2 * CH : hti * T + (h2 + 1) * 2 * CH],
                g_ps[:], AF.Gelu, bias=b1_s[:, hti : hti + 1])

    xwo_chunk(NCH - 1)
    stats_rsqrt(range(4 * NCH - 4, 4 * NCH), s2, s2i)
    ffn1_h2(0)
    for j in range(4 * NCH - 4, 4 * NCH):
        scale_transpose(j, s2i, h2t)
    ffn1_h2(1)
    for j in range(NT):
        ffn2_unit(j)
    psF_cm.__exit__(None, None, None)
    pj_cm.__exit__(None, None, None)
    pg_cm.__exit__(None, None, None)
    pout_cm.__exit__(None, None, None)
    pnrm_cm.__exit__(None, None, None)
    patt_cm.__exit__(None, None, None)
    prs_cm.__exit__(None, None, None)
    pscr_cm.__exit__(None, None, None)
    main_cm.__exit__(None, None, None)


_CACHE = {}


def _build():
    if "nc" in _CACHE:
        return _CACHE["nc"]
    nc = bacc.Bacc("TRN2", target_bir_lowering=False, debug=False)
    din = {}
    for name, shape, dt_ in (
        ("x", [T, D], F32), ("wq", [D, D], BF16), ("wk", [D, D], BF16),
        ("wv", [D, D], BF16), ("wo", [D, D], BF16), ("fw1", [D, HDIM], BF16),
        ("fb1", [HDIM], F32), ("fw2", [HDIM, D], BF16), ("fb2", [D], BF16),
    ):
        din[name] = nc.dram_tensor(name, shape, dt_, kind="ExternalInput").ap()
    out_d = nc.dram_tensor("out", [T, D], F32, kind="ExternalOutput").ap()
    with tile.TileContext(nc) as tc:
        _body(tc, din, out_d)
    nc.compile()
    _CACHE["nc"] = nc
    return nc


def run(inputs: dict, trace: bool = False):
    """Run on 8 cores; returns (output [8,T,D], BassKernelResults)."""
    nc = _build()
    x = np.ascontiguousarray(inputs["x"], dtype=np.float32)
    ln1 = np.asarray(inputs["ln1_w"], dtype=np.float32)
    ln2 = np.asarray(inputs["ln2_w"], dtype=np.float32)
    shared = {
        "wq": (ln1[:, None] * np.asarray(inputs["wq"], np.float32)).astype(ml_dtypes.bfloat16),
        "wk": (ln1[:, None] * np.asarray(inputs["wk"], np.float32)).astype(ml_dtypes.bfloat16),
        "wv": (ln1[:, None] * np.asarray(inputs["wv"], np.float32)).astype(ml_dtypes.bfloat16),
        "wo": np.asarray(inputs["wo"], np.float32).astype(ml_dtypes.bfloat16),
        "fw1": (ln2[:, None] * np.asarray(inputs["ff_w1"], np.float32)).astype(ml_dtypes.bfloat16),
        "fb1": np.asarray(inputs["ff_b1"], np.float32),
        "fw2": np.asarray(inputs["ff_w2"], np.float32).astype(ml_dtypes.bfloat16),
        "fb2": np.asarray(inputs["ff_b2"], np.float32).astype(ml_dtypes.bfloat16),
    }
    shared = {k: np.ascontiguousarray(v) for k, v in shared.items()}
    in_maps = [dict(shared, x=np.ascontiguousarray(x[c])) for c in range(NCORES)]
    res = run_bass_kernel_spmd(nc, in_maps, list(range(NCORES)), trace=trace)
    out = np.stack([res.results[c]["out"] for c in range(NCORES)], axis=0)
    return out, res


def kernel(**inputs) -> np.ndarray:
    out, _ = run(inputs, trace=False)
    return out

